# revision 26
# baseline (speedup 1.0000x reference)
"""Trainium2 Bass kernel for nn_KinematicLayer: batched forward kinematics.

Full inputs x:[524288,26] f32 -> out:[524288,51] f32.

End-to-end wall time is dominated by the axon host<->device tunnel
(~50-80 MB/s, serial), so the kernel minimizes wire bytes:
  - x is uploaded as f16 ([N,26], 27 MB instead of 55 MB);
  - the device returns only the 13 non-derivable joint positions as f16
    ([N,39], 41 MB instead of 107 MB).  Host derives the rest:
    p0 = 0, p7 = -p4, p13 = 2*p1 - p10, thorax = (p6+p8)/2.
  - the jitted executable is cached across calls, and the previous call's
    (already fetched) device output buffers are donated back as the
    custom-call output operands, so no zero-buffer upload per call.

Device compute (per core: 65536 samples, 2 chunks of 128x256): per-sample
state tracked as (R 3x3, t 3); the five limb chains share one instruction
stream batched along the free dim (FD=1280).  Trig via half-angle
identities keeps every ACT Sin argument inside the spline's valid
[-pi,pi] range: u=sin(x/2), w=sin(x/4), v=1-2w^2=cos(x/2), cos=1-2u^2,
sin=2uv.  Intermediates fp16 (DVE 2x mode).
"""
import numpy as np
import jax
from jax.sharding import Mesh, PartitionSpec
from jax.experimental.shard_map import shard_map

import concourse.bass as bass
import concourse.tile as tile
import concourse.bass2jax as b2j
from concourse import bacc, mybir

AF = mybir.ActivationFunctionType
ALU = mybir.AluOpType
f32, f16, i8 = mybir.dt.float32, mybir.dt.float16, mybir.dt.int8

N = 524288
K = 25                      # device only needs the 25 angle cols (not scale)
J = 39                      # 13 joints x 3 shipped to host
NCORE = 8
NGRP = 2                    # pipeline groups (upload/exec/download overlap)
NG = N // NGRP              # 262144 samples per group
NPC = NG // NCORE           # 32768 samples per core per group
FD = min(256, NPC // 128)   # samples per partition per chunk
CHUNK = 128 * FD            # 32768 samples per chunk
NCHUNK = NPC // CHUNK       # 1

_S = np.array([300.0, 350.0, 75.0, 400.0, 73.96, 249.03, 250.0, 250.0, 170.0],
              np.float32) / 300.0
S0, S1, S2, S3, S4, S5, S6, S7, S8 = [float(v) for v in _S]

# chain order: (neck, Lleg, Rleg, Larm, Rarm); euler angle bases 5,9,13,17,21
DT1 = [S4, -S1, -S1, -S7, -S7]   # signed first-translation lengths
DT2 = [S5, -S0, -S0, -S6, -S6]   # signed distal-translation lengths

FDC = 5 * FD                 # batched chain free dim

# Output column layout: the 13 shipped joints in final joint order
# [1,2,3,4,5,6,8,9,10,11,12,14,15], 3 cols each — so host assembly is a
# handful of wide contiguous block copies.
#   cols  0: 3 j1 torso | 3: 6 j2 | 6: 9 j3 | 9:12 j4 | 12:15 j5 | 15:18 j6
#   cols 18:21 j8 | 21:24 j9 | 24:27 j10 | 27:30 j11 | 30:33 j12
#   cols 33:36 j14 | 36:39 j15
# Knee-level joints (2,5,8,11,14) land at col bases (3,12,18,27,33):
# chains {0,2,4} -> 3,18,33 (stride 15), chains {1,3} -> 12,27 (stride 15).
# Distal joints (3,6,9,12,15) at (6,15,21,30,36): same two-group split.
#
# int8 downlink: positions are exactly linear in the scale input x[:,25],
# so the device computes unit-scale positions (bounded per joint by its
# bone-length sum), quantizes q = v*127/(bound*MARGIN) to int8, and the
# host recovers v = q*(bound*MARGIN/127)*scale.
MARGIN = 1.02
B_TOR = S3                   # |torso| = S3 exactly
B_HIP = S2                   # |hip| = S2 exactly
B_SH = S3 + S8               # shoulder
BK = [S3 + S4, S2 + S1, S2 + S1, B_SH + S7, B_SH + S7]   # knee-level
BD = [BK[0] + S5, BK[1] + S0, BK[2] + S0, BK[3] + S6, BK[4] + S6]  # distal

# host dequant vector, col -> bound*MARGIN/127
_BOUNDS = ([B_TOR] * 3 + [BK[0]] * 3 + [BD[0]] * 3 + [B_HIP] * 3 +
           [BK[1]] * 3 + [BD[1]] * 3 + [BK[2]] * 3 + [BD[2]] * 3 +
           [B_SH] * 3 + [BK[3]] * 3 + [BD[3]] * 3 + [BK[4]] * 3 +
           [BD[4]] * 3)
DEQ = (np.asarray(_BOUNDS, np.float32) * MARGIN / 127.0)


def mk(ap, off, dims):
    """Custom free-dim AP on the same tile/tensor (keeps partition dim)."""
    return bass.AP(ap.tensor, ap.offset + off, [list(ap.ap[0])] + dims)


def build():
    nc = bacc.Bacc("TRN2", target_bir_lowering=False, debug=False,
                   num_devices=NCORE)
    x = nc.dram_tensor("x", [NPC, K], f16, kind="ExternalInput").ap()
    y = nc.dram_tensor("y", [NPC, J], i8, kind="ExternalOutput").ap()

    with tile.TileContext(nc) as tc:
        with (
            tc.tile_pool(name="io", bufs=1) as io,
            tc.tile_pool(name="per", bufs=1) as per,
            tc.tile_pool(name="scr", bufs=1) as scr,
        ):
            for ch in range(NCHUNK):
                build_chunk(nc, tc, io, per, scr, x, y, ch)
    nc.compile()
    return nc


def build_chunk(nc, tc, io, per, scr, x, y, ch):
    V, A = nc.vector, nc.scalar
    base = ch * CHUNK

    X = io.tile([128, K * FD], f16, tag="X")
    HX = K * FD // 2
    for h in range(2):
        nc.gpsimd.dma_start(X[:, h * HX:(h + 1) * HX],
                            bass.AP(x.tensor, base * K + h * HX,
                                    [[FD * K, 128], [1, HX]]))
    Y = io.tile([128, J * FD], i8, tag="Y")
    Xa = X[:]
    Ya = Y[:]

    def ycol(c):                       # output scalar col c strided [128,FD]
        return mk(Ya, c, [[J, FD]])

    def ygrpA(c0):                     # chains 0,2,4 -> 3 joints stride 15
        return mk(Ya, c0, [[15, 3], [J, FD]])

    def ygrpB(c0):                     # chains 1,3 -> 2 joints stride 15
        return mk(Ya, c0, [[15, 2], [J, FD]])

    def srcA(t):                       # chain-major [128,5*FD] -> chains 0,2,4
        a = t if isinstance(t, bass.AP) else t[:]
        return bass.AP(a.tensor, a.offset, [list(a.ap[0]), [2 * FD, 3], [1, FD]])

    def srcB(t):                       # chains 1,3
        a = t if isinstance(t, bass.AP) else t[:]
        return bass.AP(a.tensor, a.offset + FD,
                       [list(a.ap[0]), [2 * FD, 2], [1, FD]])

    # ---------------- trig: 5 groups ----------------
    def trig(tag, xap, n):
        fd = n * FD
        u = scr.tile([128, fd], f16, tag="trigU", name="trigU")
        w = scr.tile([128, fd], f16, tag="trigW", name="trigW")
        A.activation(u[:], xap, AF.Sin, scale=0.5)
        A.activation(w[:], xap, AF.Sin, scale=0.25)
        q = scr.tile([128, fd], f16, tag="trigQ", name="trigQ")
        c = per.tile([128, fd], f16, tag=f"C{tag}", name=f"C{tag}")
        s = per.tile([128, fd], f16, tag=f"S{tag}", name=f"S{tag}")
        A.square(q[:], u[:])
        V.tensor_scalar(c[:], q[:], -2.0, 1.0, ALU.mult, ALU.add)
        A.square(q[:], w[:])
        V.tensor_scalar(q[:], q[:], -2.0, 1.0, ALU.mult, ALU.add)  # v in q
        V.scalar_tensor_tensor(s[:], u[:], 2.0, q[:], ALU.mult, ALU.mult)
        return c, s

    Cpt, Spt = trig("pt", mk(Xa, 0, [[1, 5], [K, FD]]), 5)
    CS = [trig(f"p{j}", mk(Xa, 5 + j, [[4, 5], [K, FD]]), 5) for j in range(4)]

    def pt(t, i):
        return t[:, i * FD:(i + 1) * FD]

    c0, s0 = pt(Cpt, 0), pt(Spt, 0)
    c1, s1 = pt(Cpt, 1), pt(Spt, 1)
    c2, s2 = pt(Cpt, 2), pt(Spt, 2)
    c3, s3 = pt(Cpt, 3), pt(Spt, 3)
    c4, s4 = pt(Cpt, 4), pt(Spt, 4)

    def tt(out, a, b, op):
        V.tensor_tensor(out, a, b, op)

    def fresh(tag, fd=FD, dt=f16, pool=None):
        return (pool or scr).tile([128, fd], dt, tag=tag, name=tag)

    def mul(a, b, tag="m", fd=FD):
        o = fresh(tag, fd=fd)
        tt(o[:], a, b, ALU.mult)
        return o[:]

    def nmul(a, b, tag="m"):           # -(a*b)
        o = fresh(tag)
        V.scalar_tensor_tensor(o[:], a, -1.0, b, ALU.mult, ALU.mult)
        return o[:]

    def comb(a, b, op, tag="m", pool=None, fd=FD):
        o = fresh(tag, fd=fd, pool=pool)
        tt(o[:], a, b, op)
        return o[:]

    # ---------------- pelvis R ----------------
    ms0s1 = mul(s0, s1, "ms01")
    mc0s1 = mul(c0, s1, "mc01")
    P1x = nmul(s0, c1, "P1x")
    P1y = mul(c0, c1, "P1y")
    P1z = s1                                        # alias
    P0x = comb(mul(c0, c2), mul(ms0s1, s2, "m2"), ALU.subtract, "P0x", per)
    P0y = comb(mul(s0, c2), mul(mc0s1, s2, "m2"), ALU.add, "P0y", per)
    P0z = nmul(c1, s2, "P0z")
    P2x = comb(mul(c0, s2), mul(ms0s1, c2, "m2"), ALU.add, "P2x", per)
    P2y = comb(mul(s0, s2), mul(mc0s1, c2, "m2"), ALU.subtract, "P2y", per)
    P2z = mul(c1, c2, "P2z")
    P0 = (P0x, P0y, P0z)
    P1 = (P1x, P1y, P1z)
    P2 = (P2x, P2y, P2z)

    # ---------------- torso R = Rpel @ Rz3 @ Ry4 ----------------
    def colupd(cc, ss, A3, B3, tagp, pool=None, fd=FD):
        """returns cc*A + ss*B per component."""
        out = []
        for i, (a, b) in enumerate(zip(A3, B3)):
            out.append(comb(mul(cc, a, "ca", fd), mul(ss, b, "cb", fd), ALU.add,
                            f"{tagp}{i}", pool, fd))
        return tuple(out)

    def colupd_sub(cc, ss, A3, B3, tagp, pool=None, fd=FD):
        """returns cc*A - ss*B per component."""
        out = []
        for i, (a, b) in enumerate(zip(A3, B3)):
            out.append(comb(mul(cc, a, "ca", fd), mul(ss, b, "cb", fd),
                            ALU.subtract, f"{tagp}{i}", pool, fd))
        return tuple(out)

    D0t = colupd(c3, s3, P0, P1, "D0t")
    D1t = colupd_sub(c3, s3, P1, P0, "D1t", per)       # E1 = D1t
    E0 = colupd_sub(c4, s4, D0t, P2, "E0", per)
    E2 = colupd(s4, c4, D0t, P2, "E2", per)

    # ---------------- phase A translations (unit scale) ----------------
    TP = [per.tile([128, FDC], f16, tag=f"TP{c}", name=f"TP{c}")
          for c in range(3)]

    def tp_slice(c, i):
        return TP[c][:, i * FD:(i + 1) * FD]

    QTOR = 127.0 / MARGIN            # S3*127/(S3*MARGIN) folded
    QHIP = 127.0 / MARGIN
    QSH = 127.0 / (B_SH * MARGIN)
    for c in range(3):
        # torso t = S3*D1 -> Y joint1 (quantized) + TP[neck]
        A.mul(ycol(0 + c), D1t[c], QTOR)             # D1t*S3*127/(S3*M)
        A.mul(tp_slice(c, 0), D1t[c], S3)
        # hips: +-S2*P0 -> TP legs; left hip -> Y
        A.mul(tp_slice(c, 1), P0[c], S2)
        A.mul(tp_slice(c, 2), P0[c], -S2)
        A.mul(ycol(9 + c), P0[c], QHIP)              # P0*S2*127/(S2*M)
        # shoulders: t_tor +- S8*E0 -> TP arms; left shoulder -> Y
        u = fresh("shu")
        A.mul(u[:], E0[c], S8)
        tt(tp_slice(c, 3), tp_slice(c, 0), u[:], ALU.add)
        tt(tp_slice(c, 4), tp_slice(c, 0), u[:], ALU.subtract)
        A.mul(ycol(24 + c), tp_slice(c, 3), QSH)

    # ---------------- batched parent-R tiles ----------------
    # chains: 0=neck(E), 1,2=legs(P), 3,4=arms(E)
    PR = [[per.tile([128, FDC], f16, tag=f"PR{c}{i}", name=f"PR{c}{i}")
           for i in range(3)] for c in range(3)]
    for ci, (Ecol, Pcol) in enumerate(((E0, P0), (D1t, P1), (E2, P2))):
        for i in range(3):
            dst = PR[ci][i][:]
            e = Ecol[i]
            p = Pcol[i]

            def bc2(src):
                return bass.AP(src.tensor, src.offset,
                               [list(src.ap[0]), [0, 2], [1, FD]])

            A.copy(mk(dst, 0, [[1, FD]]), e)
            A.copy(mk(dst, FD, [[1, 2 * FD]]), bc2(p))
            A.copy(mk(dst, 3 * FD, [[1, 2 * FD]]), bc2(e))

    def prc(c):
        return tuple(PR[c][i][:] for i in range(3))

    cA, sA = (t[:] for t in CS[0])
    cB, sB = (t[:] for t in CS[1])
    cG, sG = (t[:] for t in CS[2])
    cD, sD = (t[:] for t in CS[3])

    # ---------------- batched chain (FD=1280 ops) ----------------
    bD0 = colupd(cA, sA, prc(0), prc(1), "bD0", per, FDC)
    bD1 = colupd_sub(cA, sA, prc(1), prc(0), "bD1", per, FDC)
    bK1 = colupd(cB, sB, bD1, prc(2), "bK1", per, FDC)
    bK2 = colupd_sub(cB, sB, prc(2), bD1, "bK2", per, FDC)
    bK2p = colupd(sG, cG, bD0, bK2, "bD1", per, FDC)  # reuse bD1 slots
    bC1 = colupd(cD, sD, bK1, bK2p, "bD0", per, FDC)  # reuse bD0 slots

    # constant tiles: per-chain signed bone lengths and quant scales
    dT1 = fresh("dT1", FDC, pool=per)
    dT2 = fresh("dT2", FDC, pool=per)
    Qk = fresh("Qk", FDC, pool=per)
    Qd = fresh("Qd", FDC, pool=per)
    for i in range(5):
        sl = slice(i * FD, (i + 1) * FD)
        V.memset(dT1[:, sl], DT1[i])
        V.memset(dT2[:, sl], DT2[i])
        V.memset(Qk[:, sl], 127.0 / (BK[i] * MARGIN))
        V.memset(Qd[:, sl], 127.0 / (BD[i] * MARGIN))

    for c in range(3):
        u = fresh("btr", FDC)
        tt(u[:], dT1[:], bK1[c], ALU.mult)
        kn = fresh("kn", FDC)
        tt(kn[:], TP[c][:], u[:], ALU.add)               # knee-level joints
        u2 = fresh("btr2", FDC)
        tt(u2[:], dT2[:], bC1[c], ALU.mult)
        ds = fresh("ds", FDC)
        tt(ds[:], kn[:], u2[:], ALU.add)                 # distal joints
        knq = fresh("knq", FDC)
        tt(knq[:], kn[:], Qk[:], ALU.mult)               # quantize
        dsq = fresh("dsq", FDC)
        tt(dsq[:], ds[:], Qd[:], ALU.mult)
        A.copy(ygrpA(3 + c), srcA(knq))
        A.copy(ygrpB(12 + c), srcB(knq))
        A.copy(ygrpA(6 + c), srcA(dsq))
        A.copy(ygrpB(15 + c), srcB(dsq))

    HY = J * FD // 2
    for h in range(2):
        nc.gpsimd.dma_start(bass.AP(y.tensor, base * J + h * HY,
                                    [[FD * J, 128], [1, HY]]),
                            Y[:, h * HY:(h + 1) * HY])


# ---------------------------------------------------------------------------
# Cached PJRT runner: jit(shard_map(bass_exec)) built once; the previous
# call's device output buffers (already copied to host) are donated back as
# the custom-call output operands, so steady-state wire traffic is just
# x (f16 up) + y (f16 down).
# ---------------------------------------------------------------------------
_STATE = None


def _init():
    nc = build()
    b2j.install_neuronx_cc_hook()

    partition_name = (nc.partition_id_tensor.name
                      if nc.partition_id_tensor else None)
    in_names, out_names, out_avals = [], [], []
    for alloc in nc.m.functions[0].allocations:
        if not isinstance(alloc, mybir.MemoryLocationSet):
            continue
        name = alloc.memorylocations[0].name
        if alloc.kind == "ExternalInput":
            if name != partition_name:
                in_names.append(name)
        elif alloc.kind == "ExternalOutput":
            out_names.append(name)
            out_avals.append(jax.core.ShapedArray(
                tuple(alloc.tensor_shape), mybir.dt.np(alloc.dtype)))
    assert in_names == ["x"] and out_names == ["y"], (in_names, out_names)
    n_params = len(in_names)
    in_names_all = in_names + out_names
    if partition_name is not None:
        in_names_all.append(partition_name)
    donate = tuple(range(n_params, n_params + len(out_names)))

    def _body(*args):
        operands = list(args)
        if partition_name is not None:
            operands.append(b2j.partition_id_tensor())
        outs = b2j._bass_exec_p.bind(
            *operands,
            out_avals=tuple(out_avals),
            in_names=tuple(in_names_all),
            out_names=tuple(out_names),
            lowering_input_output_aliases=(),
            sim_require_finite=True,
            sim_require_nnan=True,
            nc=nc,
        )
        return tuple(outs)

    devices = jax.devices()[:NCORE]
    assert len(devices) == NCORE
    mesh = Mesh(np.asarray(devices), ("core",))
    nin = n_params + len(out_names)
    fn = jax.jit(
        shard_map(_body, mesh=mesh,
                  in_specs=(PartitionSpec("core"),) * nin,
                  out_specs=(PartitionSpec("core"),) * len(out_names),
                  check_rep=False),
        donate_argnums=donate,
        keep_unused=True,
    )
    return {"fn": fn, "prev": None}


def _assemble(res, y8, scl):
    """Dequantize shipped [*,39] int8 block into final [*,51] f32 rows."""
    B = y8.astype(np.float32)
    B *= DEQ[None, :]
    B *= scl[:, None]
    res[:, 0:3] = 0.0                                   # pelvis
    res[:, 3:21] = B[:, 0:18]                           # j1..j6
    res[:, 24:30] = B[:, 18:24]                         # j8, j9
    res[:, 30:39] = B[:, 24:33]                         # j10, j11, j12
    res[:, 42:48] = B[:, 33:39]                         # j14, j15
    res[:, 21:24] = -B[:, 9:12]                         # rhip = -lhip
    res[:, 39:42] = 2.0 * B[:, 0:3] - B[:, 24:27]       # rsh = 2*torso - lsh
    res[:, 48:51] = 0.5 * (B[:, 15:18] + B[:, 18:21])   # thorax = (j6+j8)/2


def kernel(x: np.ndarray) -> np.ndarray:
    global _STATE
    if _STATE is None:
        _STATE = _init()
    st = _STATE

    x = np.asarray(x)
    scl = np.ascontiguousarray(x[:, 25], dtype=np.float32)
    if st["prev"] is None:
        st["prev"] = [np.zeros((NG, J), np.int8) for _ in range(NGRP)]

    # Dispatch group g, converting group g+1's input while g uploads and
    # queueing g's device->host copies right away.
    outs = []
    all_datas = []
    for g in range(NGRP):
        xg16 = x[g * NG:(g + 1) * NG, :K].astype(np.float16)
        out, = st["fn"](xg16, st["prev"][g])
        outs.append(out)
        shards = sorted(out.addressable_shards,
                        key=lambda s: s.index[0].start or 0)
        datas = [s.data for s in shards]
        all_datas.extend(datas)
        for d in datas:
            try:
                d.copy_to_host_async()
            except Exception:
                pass

    # Assemble each shard's rows while later shards are still on the wire.
    res = np.empty((N, 51), np.float32)
    r0 = 0
    for d in all_datas:
        y8 = np.asarray(d)
        r1 = r0 + y8.shape[0]
        _assemble(res[r0:r1], y8, scl[r0:r1])
        r0 = r1
    assert r0 == N
    st["prev"] = outs                    # donate next call (already fetched)
    return res


# revision 29
# speedup vs baseline: 1.0867x; 1.0867x over previous
"""Trainium2 Bass kernel for nn_KinematicLayer: batched forward kinematics.

Full inputs x:[524288,26] f32 -> out:[524288,51] f32.

End-to-end wall time is dominated by the axon host<->device tunnel
(~50-80 MB/s, serial), so the kernel minimizes wire bytes:
  - x is uploaded as f16 ([N,26], 27 MB instead of 55 MB);
  - the device returns only the 13 non-derivable joint positions as f16
    ([N,39], 41 MB instead of 107 MB).  Host derives the rest:
    p0 = 0, p7 = -p4, p13 = 2*p1 - p10, thorax = (p6+p8)/2.
  - the jitted executable is cached across calls, and the previous call's
    (already fetched) device output buffers are donated back as the
    custom-call output operands, so no zero-buffer upload per call.

Device compute (per core: 65536 samples, 2 chunks of 128x256): per-sample
state tracked as (R 3x3, t 3); the five limb chains share one instruction
stream batched along the free dim (FD=1280).  Trig via half-angle
identities keeps every ACT Sin argument inside the spline's valid
[-pi,pi] range: u=sin(x/2), w=sin(x/4), v=1-2w^2=cos(x/2), cos=1-2u^2,
sin=2uv.  Intermediates fp16 (DVE 2x mode).
"""
import numpy as np
import jax
from jax.sharding import Mesh, PartitionSpec
from jax.experimental.shard_map import shard_map

import concourse.bass as bass
import concourse.tile as tile
import concourse.bass2jax as b2j
from concourse import bacc, mybir

AF = mybir.ActivationFunctionType
ALU = mybir.AluOpType
f32, f16, i8 = mybir.dt.float32, mybir.dt.float16, mybir.dt.int8

N = 524288
K = 25                      # device only needs the 25 angle cols (not scale)
J = 39                      # 13 joints x 3 shipped to host
NCORE = 8
NGRP = 2                    # pipeline groups (upload/exec/download overlap)
NG = N // NGRP              # 262144 samples per group
NPC = NG // NCORE           # 32768 samples per core per group
FD = min(256, NPC // 128)   # samples per partition per chunk
CHUNK = 128 * FD            # 32768 samples per chunk
NCHUNK = NPC // CHUNK       # 1

_S = np.array([300.0, 350.0, 75.0, 400.0, 73.96, 249.03, 250.0, 250.0, 170.0],
              np.float32) / 300.0
S0, S1, S2, S3, S4, S5, S6, S7, S8 = [float(v) for v in _S]

# chain order: (neck, Lleg, Rleg, Larm, Rarm); euler angle bases 5,9,13,17,21
DT1 = [S4, -S1, -S1, -S7, -S7]   # signed first-translation lengths
DT2 = [S5, -S0, -S0, -S6, -S6]   # signed distal-translation lengths

FDC = 5 * FD                 # batched chain free dim

# Output column layout: the 13 shipped joints in final joint order
# [1,2,3,4,5,6,8,9,10,11,12,14,15], 3 cols each — so host assembly is a
# handful of wide contiguous block copies.
#   cols  0: 3 j1 torso | 3: 6 j2 | 6: 9 j3 | 9:12 j4 | 12:15 j5 | 15:18 j6
#   cols 18:21 j8 | 21:24 j9 | 24:27 j10 | 27:30 j11 | 30:33 j12
#   cols 33:36 j14 | 36:39 j15
# Knee-level joints (2,5,8,11,14) land at col bases (3,12,18,27,33):
# chains {0,2,4} -> 3,18,33 (stride 15), chains {1,3} -> 12,27 (stride 15).
# Distal joints (3,6,9,12,15) at (6,15,21,30,36): same two-group split.
#
# int8 downlink: positions are exactly linear in the scale input x[:,25],
# so the device computes unit-scale positions (bounded per joint by its
# bone-length sum), quantizes q = v*127/(bound*MARGIN) to int8, and the
# host recovers v = q*(bound*MARGIN/127)*scale.
MARGIN = 1.02
B_TOR = S3                   # |torso| = S3 exactly
B_HIP = S2                   # |hip| = S2 exactly
B_SH = S3 + S8               # shoulder
BK = [S3 + S4, S2 + S1, S2 + S1, B_SH + S7, B_SH + S7]   # knee-level
BD = [BK[0] + S5, BK[1] + S0, BK[2] + S0, BK[3] + S6, BK[4] + S6]  # distal

# host dequant vector, col -> bound*MARGIN/127
_BOUNDS = ([B_TOR] * 3 + [BK[0]] * 3 + [BD[0]] * 3 + [B_HIP] * 3 +
           [BK[1]] * 3 + [BD[1]] * 3 + [BK[2]] * 3 + [BD[2]] * 3 +
           [B_SH] * 3 + [BK[3]] * 3 + [BD[3]] * 3 + [BK[4]] * 3 +
           [BD[4]] * 3)
DEQ = (np.asarray(_BOUNDS, np.float32) * MARGIN / 127.0)


def mk(ap, off, dims):
    """Custom free-dim AP on the same tile/tensor (keeps partition dim)."""
    return bass.AP(ap.tensor, ap.offset + off, [list(ap.ap[0])] + dims)


def build():
    nc = bacc.Bacc("TRN2", target_bir_lowering=False, debug=False,
                   num_devices=NCORE)
    x = nc.dram_tensor("x", [NPC, K], f16, kind="ExternalInput").ap()
    y = nc.dram_tensor("y", [NPC, J], i8, kind="ExternalOutput").ap()

    with tile.TileContext(nc) as tc:
        with (
            tc.tile_pool(name="io", bufs=1) as io,
            tc.tile_pool(name="per", bufs=1) as per,
            tc.tile_pool(name="scr", bufs=1) as scr,
        ):
            for ch in range(NCHUNK):
                build_chunk(nc, tc, io, per, scr, x, y, ch)
    nc.compile()
    return nc


def build_chunk(nc, tc, io, per, scr, x, y, ch):
    V, A = nc.vector, nc.scalar
    base = ch * CHUNK

    X = io.tile([128, K * FD], f16, tag="X")
    HX = K * FD // 2
    for h in range(2):
        nc.gpsimd.dma_start(X[:, h * HX:(h + 1) * HX],
                            bass.AP(x.tensor, base * K + h * HX,
                                    [[FD * K, 128], [1, HX]]))
    Y = io.tile([128, J * FD], i8, tag="Y")
    Xa = X[:]
    Ya = Y[:]

    def ycol(c):                       # output scalar col c strided [128,FD]
        return mk(Ya, c, [[J, FD]])

    def ygrpA(c0):                     # chains 0,2,4 -> 3 joints stride 15
        return mk(Ya, c0, [[15, 3], [J, FD]])

    def ygrpB(c0):                     # chains 1,3 -> 2 joints stride 15
        return mk(Ya, c0, [[15, 2], [J, FD]])

    def srcA(t):                       # chain-major [128,5*FD] -> chains 0,2,4
        a = t if isinstance(t, bass.AP) else t[:]
        return bass.AP(a.tensor, a.offset, [list(a.ap[0]), [2 * FD, 3], [1, FD]])

    def srcB(t):                       # chains 1,3
        a = t if isinstance(t, bass.AP) else t[:]
        return bass.AP(a.tensor, a.offset + FD,
                       [list(a.ap[0]), [2 * FD, 2], [1, FD]])

    # ---------------- trig: 5 groups ----------------
    def trig(tag, xap, n):
        fd = n * FD
        u = scr.tile([128, fd], f16, tag="trigU", name="trigU")
        w = scr.tile([128, fd], f16, tag="trigW", name="trigW")
        A.activation(u[:], xap, AF.Sin, scale=0.5)
        A.activation(w[:], xap, AF.Sin, scale=0.25)
        q = scr.tile([128, fd], f16, tag="trigQ", name="trigQ")
        c = per.tile([128, fd], f16, tag=f"C{tag}", name=f"C{tag}")
        s = per.tile([128, fd], f16, tag=f"S{tag}", name=f"S{tag}")
        A.square(q[:], u[:])
        V.tensor_scalar(c[:], q[:], -2.0, 1.0, ALU.mult, ALU.add)
        A.square(q[:], w[:])
        V.tensor_scalar(q[:], q[:], -2.0, 1.0, ALU.mult, ALU.add)  # v in q
        V.scalar_tensor_tensor(s[:], u[:], 2.0, q[:], ALU.mult, ALU.mult)
        return c, s

    Cpt, Spt = trig("pt", mk(Xa, 0, [[1, 5], [K, FD]]), 5)
    CS = [trig(f"p{j}", mk(Xa, 5 + j, [[4, 5], [K, FD]]), 5) for j in range(4)]

    def pt(t, i):
        return t[:, i * FD:(i + 1) * FD]

    c0, s0 = pt(Cpt, 0), pt(Spt, 0)
    c1, s1 = pt(Cpt, 1), pt(Spt, 1)
    c2, s2 = pt(Cpt, 2), pt(Spt, 2)
    c3, s3 = pt(Cpt, 3), pt(Spt, 3)
    c4, s4 = pt(Cpt, 4), pt(Spt, 4)

    def tt(out, a, b, op):
        V.tensor_tensor(out, a, b, op)

    def fresh(tag, fd=FD, dt=f16, pool=None):
        return (pool or scr).tile([128, fd], dt, tag=tag, name=tag)

    def mul(a, b, tag="m", fd=FD):
        o = fresh(tag, fd=fd)
        tt(o[:], a, b, ALU.mult)
        return o[:]

    def nmul(a, b, tag="m"):           # -(a*b)
        o = fresh(tag)
        V.scalar_tensor_tensor(o[:], a, -1.0, b, ALU.mult, ALU.mult)
        return o[:]

    def comb(a, b, op, tag="m", pool=None, fd=FD):
        o = fresh(tag, fd=fd, pool=pool)
        tt(o[:], a, b, op)
        return o[:]

    # ---------------- pelvis R ----------------
    ms0s1 = mul(s0, s1, "ms01")
    mc0s1 = mul(c0, s1, "mc01")
    P1x = nmul(s0, c1, "P1x")
    P1y = mul(c0, c1, "P1y")
    P1z = s1                                        # alias
    P0x = comb(mul(c0, c2), mul(ms0s1, s2, "m2"), ALU.subtract, "P0x", per)
    P0y = comb(mul(s0, c2), mul(mc0s1, s2, "m2"), ALU.add, "P0y", per)
    P0z = nmul(c1, s2, "P0z")
    P2x = comb(mul(c0, s2), mul(ms0s1, c2, "m2"), ALU.add, "P2x", per)
    P2y = comb(mul(s0, s2), mul(mc0s1, c2, "m2"), ALU.subtract, "P2y", per)
    P2z = mul(c1, c2, "P2z")
    P0 = (P0x, P0y, P0z)
    P1 = (P1x, P1y, P1z)
    P2 = (P2x, P2y, P2z)

    # ---------------- torso R = Rpel @ Rz3 @ Ry4 ----------------
    def colupd(cc, ss, A3, B3, tagp, pool=None, fd=FD):
        """returns cc*A + ss*B per component."""
        out = []
        for i, (a, b) in enumerate(zip(A3, B3)):
            out.append(comb(mul(cc, a, "ca", fd), mul(ss, b, "cb", fd), ALU.add,
                            f"{tagp}{i}", pool, fd))
        return tuple(out)

    def colupd_sub(cc, ss, A3, B3, tagp, pool=None, fd=FD):
        """returns cc*A - ss*B per component."""
        out = []
        for i, (a, b) in enumerate(zip(A3, B3)):
            out.append(comb(mul(cc, a, "ca", fd), mul(ss, b, "cb", fd),
                            ALU.subtract, f"{tagp}{i}", pool, fd))
        return tuple(out)

    D0t = colupd(c3, s3, P0, P1, "D0t")
    D1t = colupd_sub(c3, s3, P1, P0, "D1t", per)       # E1 = D1t
    E0 = colupd_sub(c4, s4, D0t, P2, "E0", per)
    E2 = colupd(s4, c4, D0t, P2, "E2", per)

    # ---------------- phase A translations (unit scale) ----------------
    TP = [per.tile([128, FDC], f16, tag=f"TP{c}", name=f"TP{c}")
          for c in range(3)]

    def tp_slice(c, i):
        return TP[c][:, i * FD:(i + 1) * FD]

    QTOR = 127.0 / MARGIN            # S3*127/(S3*MARGIN) folded
    QHIP = 127.0 / MARGIN
    QSH = 127.0 / (B_SH * MARGIN)
    for c in range(3):
        # torso t = S3*D1 -> Y joint1 (quantized) + TP[neck]
        A.mul(ycol(0 + c), D1t[c], QTOR)             # D1t*S3*127/(S3*M)
        A.mul(tp_slice(c, 0), D1t[c], S3)
        # hips: +-S2*P0 -> TP legs; left hip -> Y
        A.mul(tp_slice(c, 1), P0[c], S2)
        A.mul(tp_slice(c, 2), P0[c], -S2)
        A.mul(ycol(9 + c), P0[c], QHIP)              # P0*S2*127/(S2*M)
        # shoulders: t_tor +- S8*E0 -> TP arms; left shoulder -> Y
        u = fresh("shu")
        A.mul(u[:], E0[c], S8)
        tt(tp_slice(c, 3), tp_slice(c, 0), u[:], ALU.add)
        tt(tp_slice(c, 4), tp_slice(c, 0), u[:], ALU.subtract)
        A.mul(ycol(24 + c), tp_slice(c, 3), QSH)

    # ---------------- batched parent-R tiles ----------------
    # chains: 0=neck(E), 1,2=legs(P), 3,4=arms(E)
    PR = [[per.tile([128, FDC], f16, tag=f"PR{c}{i}", name=f"PR{c}{i}")
           for i in range(3)] for c in range(3)]
    for ci, (Ecol, Pcol) in enumerate(((E0, P0), (D1t, P1), (E2, P2))):
        for i in range(3):
            dst = PR[ci][i][:]
            e = Ecol[i]
            p = Pcol[i]

            def bc2(src):
                return bass.AP(src.tensor, src.offset,
                               [list(src.ap[0]), [0, 2], [1, FD]])

            A.copy(mk(dst, 0, [[1, FD]]), e)
            A.copy(mk(dst, FD, [[1, 2 * FD]]), bc2(p))
            A.copy(mk(dst, 3 * FD, [[1, 2 * FD]]), bc2(e))

    def prc(c):
        return tuple(PR[c][i][:] for i in range(3))

    cA, sA = (t[:] for t in CS[0])
    cB, sB = (t[:] for t in CS[1])
    cG, sG = (t[:] for t in CS[2])
    cD, sD = (t[:] for t in CS[3])

    # ---------------- batched chain (FD=1280 ops) ----------------
    bD0 = colupd(cA, sA, prc(0), prc(1), "bD0", per, FDC)
    bD1 = colupd_sub(cA, sA, prc(1), prc(0), "bD1", per, FDC)
    bK1 = colupd(cB, sB, bD1, prc(2), "bK1", per, FDC)
    bK2 = colupd_sub(cB, sB, prc(2), bD1, "bK2", per, FDC)
    bK2p = colupd(sG, cG, bD0, bK2, "bD1", per, FDC)  # reuse bD1 slots
    bC1 = colupd(cD, sD, bK1, bK2p, "bD0", per, FDC)  # reuse bD0 slots

    # constant tiles: per-chain signed bone lengths and quant scales
    dT1 = fresh("dT1", FDC, pool=per)
    dT2 = fresh("dT2", FDC, pool=per)
    Qk = fresh("Qk", FDC, pool=per)
    Qd = fresh("Qd", FDC, pool=per)
    for i in range(5):
        sl = slice(i * FD, (i + 1) * FD)
        V.memset(dT1[:, sl], DT1[i])
        V.memset(dT2[:, sl], DT2[i])
        V.memset(Qk[:, sl], 127.0 / (BK[i] * MARGIN))
        V.memset(Qd[:, sl], 127.0 / (BD[i] * MARGIN))

    for c in range(3):
        u = fresh("btr", FDC)
        tt(u[:], dT1[:], bK1[c], ALU.mult)
        kn = fresh("kn", FDC)
        tt(kn[:], TP[c][:], u[:], ALU.add)               # knee-level joints
        u2 = fresh("btr2", FDC)
        tt(u2[:], dT2[:], bC1[c], ALU.mult)
        ds = fresh("ds", FDC)
        tt(ds[:], kn[:], u2[:], ALU.add)                 # distal joints
        knq = fresh("knq", FDC)
        tt(knq[:], kn[:], Qk[:], ALU.mult)               # quantize
        dsq = fresh("dsq", FDC)
        tt(dsq[:], ds[:], Qd[:], ALU.mult)
        A.copy(ygrpA(3 + c), srcA(knq))
        A.copy(ygrpB(12 + c), srcB(knq))
        A.copy(ygrpA(6 + c), srcA(dsq))
        A.copy(ygrpB(15 + c), srcB(dsq))

    HY = J * FD // 2
    for h in range(2):
        nc.gpsimd.dma_start(bass.AP(y.tensor, base * J + h * HY,
                                    [[FD * J, 128], [1, HY]]),
                            Y[:, h * HY:(h + 1) * HY])


# ---------------------------------------------------------------------------
# Cached PJRT runner: jit(shard_map(bass_exec)) built once; the previous
# call's device output buffers (already copied to host) are donated back as
# the custom-call output operands, so steady-state wire traffic is just
# x (f16 up) + y (f16 down).
# ---------------------------------------------------------------------------
_STATE = None


def _init():
    nc = build()
    b2j.install_neuronx_cc_hook()

    partition_name = (nc.partition_id_tensor.name
                      if nc.partition_id_tensor else None)
    in_names, out_names, out_avals = [], [], []
    for alloc in nc.m.functions[0].allocations:
        if not isinstance(alloc, mybir.MemoryLocationSet):
            continue
        name = alloc.memorylocations[0].name
        if alloc.kind == "ExternalInput":
            if name != partition_name:
                in_names.append(name)
        elif alloc.kind == "ExternalOutput":
            out_names.append(name)
            out_avals.append(jax.core.ShapedArray(
                tuple(alloc.tensor_shape), mybir.dt.np(alloc.dtype)))
    assert in_names == ["x"] and out_names == ["y"], (in_names, out_names)
    n_params = len(in_names)
    in_names_all = in_names + out_names
    if partition_name is not None:
        in_names_all.append(partition_name)
    donate = tuple(range(n_params, n_params + len(out_names)))

    def _body(*args):
        operands = list(args)
        if partition_name is not None:
            operands.append(b2j.partition_id_tensor())
        outs = b2j._bass_exec_p.bind(
            *operands,
            out_avals=tuple(out_avals),
            in_names=tuple(in_names_all),
            out_names=tuple(out_names),
            lowering_input_output_aliases=(),
            sim_require_finite=True,
            sim_require_nnan=True,
            nc=nc,
        )
        return tuple(outs)

    devices = jax.devices()[:NCORE]
    assert len(devices) == NCORE
    mesh = Mesh(np.asarray(devices), ("core",))
    nin = n_params + len(out_names)
    fn = jax.jit(
        shard_map(_body, mesh=mesh,
                  in_specs=(PartitionSpec("core"),) * nin,
                  out_specs=(PartitionSpec("core"),) * len(out_names),
                  check_rep=False),
        donate_argnums=donate,
        keep_unused=True,
    )
    # AOT-compile to trim per-call dispatch overhead.
    try:
        fn = fn.lower(jax.ShapeDtypeStruct((NG, K), np.float16),
                      jax.ShapeDtypeStruct((NG, J), np.int8)).compile()
    except Exception:
        pass
    return {"fn": fn, "prev": None}


def _assemble(res, y8, scl):
    """Dequantize shipped [*,39] int8 block into final [*,51] f32 rows."""
    B = y8.astype(np.float32)
    B *= DEQ[None, :]
    B *= scl[:, None]
    res[:, 0:3] = 0.0                                   # pelvis
    res[:, 3:21] = B[:, 0:18]                           # j1..j6
    res[:, 24:30] = B[:, 18:24]                         # j8, j9
    res[:, 30:39] = B[:, 24:33]                         # j10, j11, j12
    res[:, 42:48] = B[:, 33:39]                         # j14, j15
    res[:, 21:24] = -B[:, 9:12]                         # rhip = -lhip
    res[:, 39:42] = 2.0 * B[:, 0:3] - B[:, 24:27]       # rsh = 2*torso - lsh
    res[:, 48:51] = 0.5 * (B[:, 15:18] + B[:, 18:21])   # thorax = (j6+j8)/2


def kernel(x: np.ndarray) -> np.ndarray:
    global _STATE
    if _STATE is None:
        _STATE = _init()
    st = _STATE

    x = np.asarray(x)
    if st["prev"] is None:
        st["prev"] = [np.zeros((NG, J), np.int8) for _ in range(NGRP)]

    # Dispatch group g, converting group g+1's input while g uploads and
    # queueing g's device->host copies right away.
    outs = []
    all_datas = []
    for g in range(NGRP):
        xg16 = x[g * NG:(g + 1) * NG, :K].astype(np.float16)
        out, = st["fn"](xg16, st["prev"][g])
        outs.append(out)
        shards = sorted(out.addressable_shards,
                        key=lambda s: s.index[0].start or 0)
        datas = [s.data for s in shards]
        all_datas.extend(datas)
        for d in datas:
            try:
                d.copy_to_host_async()
            except Exception:
                pass

    # Assemble each shard's rows while later shards are still on the wire.
    scl = np.ascontiguousarray(x[:, 25], dtype=np.float32)
    res = np.empty((N, 51), np.float32)
    r0 = 0
    for d in all_datas:
        y8 = np.asarray(d)
        r1 = r0 + y8.shape[0]
        _assemble(res[r0:r1], y8, scl[r0:r1])
        r0 = r1
    assert r0 == N
    st["prev"] = outs                    # donate next call (already fetched)
    return res


# revision 37
# speedup vs baseline: 1.1224x; 1.0328x over previous
"""Trainium2 Bass kernel for nn_KinematicLayer: batched forward kinematics.

Full inputs x:[524288,26] f32 -> out:[524288,51] f32.

End-to-end wall time is dominated by the axon host<->device tunnel
(~50-80 MB/s, serial), so the kernel minimizes wire bytes:
  - x is uploaded as f16 ([N,26], 27 MB instead of 55 MB);
  - the device returns only the 13 non-derivable joint positions as f16
    ([N,39], 41 MB instead of 107 MB).  Host derives the rest:
    p0 = 0, p7 = -p4, p13 = 2*p1 - p10, thorax = (p6+p8)/2.
  - the jitted executable is cached across calls, and the previous call's
    (already fetched) device output buffers are donated back as the
    custom-call output operands, so no zero-buffer upload per call.

Device compute (per core: 65536 samples, 2 chunks of 128x256): per-sample
state tracked as (R 3x3, t 3); the five limb chains share one instruction
stream batched along the free dim (FD=1280).  Trig via half-angle
identities keeps every ACT Sin argument inside the spline's valid
[-pi,pi] range: u=sin(x/2), w=sin(x/4), v=1-2w^2=cos(x/2), cos=1-2u^2,
sin=2uv.  Intermediates fp16 (DVE 2x mode).
"""
import numpy as np
import jax
from jax.sharding import Mesh, PartitionSpec
from jax.experimental.shard_map import shard_map

import concourse.bass as bass
import concourse.tile as tile
import concourse.bass2jax as b2j
from concourse import bacc, mybir

AF = mybir.ActivationFunctionType
ALU = mybir.AluOpType
f32, f16, i8, u8 = (mybir.dt.float32, mybir.dt.float16, mybir.dt.int8,
                    mybir.dt.uint8)

N = 524288
K = 25                      # device only needs the 25 angle cols (not scale)
KP = 38                     # packed 12-bit upload bytes/sample: 25 msb +
                            # 12 nibble-pairs + 1 lone nibble
STEP = 11.264 / 4096.0      # 12-bit angle quantization step over [-5.632,5.632)
ABIAS = 5.632
J = 39                      # 13 joints x 3 shipped to host
NCORE = 8
NGRP = 2                    # pipeline groups (upload/exec/download overlap)
NG = N // NGRP              # 262144 samples per group
NPC = NG // NCORE           # 32768 samples per core per group
FD = min(256, NPC // 128)   # samples per partition per chunk
CHUNK = 128 * FD            # 32768 samples per chunk
NCHUNK = NPC // CHUNK       # 1

_S = np.array([300.0, 350.0, 75.0, 400.0, 73.96, 249.03, 250.0, 250.0, 170.0],
              np.float32) / 300.0
S0, S1, S2, S3, S4, S5, S6, S7, S8 = [float(v) for v in _S]

# chain order: (neck, Lleg, Rleg, Larm, Rarm); euler angle bases 5,9,13,17,21
DT1 = [S4, -S1, -S1, -S7, -S7]   # signed first-translation lengths
DT2 = [S5, -S0, -S0, -S6, -S6]   # signed distal-translation lengths

FDC = 5 * FD                 # batched chain free dim

# Output column layout: the 13 shipped joints in final joint order
# [1,2,3,4,5,6,8,9,10,11,12,14,15], 3 cols each — so host assembly is a
# handful of wide contiguous block copies.
#   cols  0: 3 j1 torso | 3: 6 j2 | 6: 9 j3 | 9:12 j4 | 12:15 j5 | 15:18 j6
#   cols 18:21 j8 | 21:24 j9 | 24:27 j10 | 27:30 j11 | 30:33 j12
#   cols 33:36 j14 | 36:39 j15
# Knee-level joints (2,5,8,11,14) land at col bases (3,12,18,27,33):
# chains {0,2,4} -> 3,18,33 (stride 15), chains {1,3} -> 12,27 (stride 15).
# Distal joints (3,6,9,12,15) at (6,15,21,30,36): same two-group split.
#
# int8 downlink: positions are exactly linear in the scale input x[:,25],
# so the device computes unit-scale positions (bounded per joint by its
# bone-length sum), quantizes q = v*127/(bound*MARGIN) to int8, and the
# host recovers v = q*(bound*MARGIN/127)*scale.
MARGIN = 1.02
B_TOR = S3                   # |torso| = S3 exactly
B_HIP = S2                   # |hip| = S2 exactly
B_SH = S3 + S8               # shoulder
BK = [S3 + S4, S2 + S1, S2 + S1, B_SH + S7, B_SH + S7]   # knee-level
BD = [BK[0] + S5, BK[1] + S0, BK[2] + S0, BK[3] + S6, BK[4] + S6]  # distal

# host dequant vector, col -> bound*MARGIN/127
_BOUNDS = ([B_TOR] * 3 + [BK[0]] * 3 + [BD[0]] * 3 + [B_HIP] * 3 +
           [BK[1]] * 3 + [BD[1]] * 3 + [BK[2]] * 3 + [BD[2]] * 3 +
           [B_SH] * 3 + [BK[3]] * 3 + [BD[3]] * 3 + [BK[4]] * 3 +
           [BD[4]] * 3)
DEQ = (np.asarray(_BOUNDS, np.float32) * MARGIN / 127.0)


def mk(ap, off, dims):
    """Custom free-dim AP on the same tile/tensor (keeps partition dim)."""
    return bass.AP(ap.tensor, ap.offset + off, [list(ap.ap[0])] + dims)


def build():
    nc = bacc.Bacc("TRN2", target_bir_lowering=False, debug=False,
                   num_devices=NCORE)
    x = nc.dram_tensor("x", [NPC, KP], u8, kind="ExternalInput").ap()
    y = nc.dram_tensor("y", [NPC, J], i8, kind="ExternalOutput").ap()

    with tile.TileContext(nc) as tc:
        with (
            tc.tile_pool(name="io", bufs=1) as io,
            tc.tile_pool(name="per", bufs=1) as per,
            tc.tile_pool(name="scr", bufs=1) as scr,
        ):
            for ch in range(NCHUNK):
                build_chunk(nc, tc, io, per, scr, x, y, ch)
    nc.compile()
    return nc


def build_chunk(nc, tc, io, per, scr, x, y, ch):
    V, A = nc.vector, nc.scalar
    base = ch * CHUNK

    X8 = io.tile([128, KP * FD], u8, tag="X8")
    HX = KP * FD // 2
    for h in range(2):
        nc.gpsimd.dma_start(X8[:, h * HX:(h + 1) * HX],
                            bass.AP(x.tensor, base * KP + h * HX,
                                    [[FD * KP, 128], [1, HX]]))
    Y = io.tile([128, J * FD], i8, tag="Y")
    X8a = X8[:]
    Ya = Y[:]

    # ---- unpack 12-bit angles -> X [128, K*FD] f16 (sample-major) ----
    # byte c of sample s sits at X8[p, s*KP + c]; angle k of sample s goes
    # to X[p, s*K + k].  msb byte c=k (k<25); low nibbles of c=25+i carry
    # angle 2i (i<=12, col 37's pair-high is zero), high nibbles of c=25+i
    # carry angle 2i+1 (i<12).
    X = io.tile([128, K * FD], f16, tag="X")
    Xa = X[:]
    V.tensor_scalar(Xa, bass.AP(X8a.tensor, X8a.offset,
                                [list(X8a.ap[0]), [KP, FD], [1, K]]),
                    16.0 * STEP, -ABIAS, ALU.mult, ALU.add)
    nbE = scr.tile([128, 13 * FD], u8, tag="nbE", name="nbE")
    nbO = scr.tile([128, 12 * FD], u8, tag="nbO", name="nbO")
    V.tensor_scalar(nbE[:], bass.AP(X8a.tensor, X8a.offset + 25,
                                    [list(X8a.ap[0]), [KP, FD], [1, 13]]),
                    15, None, ALU.bitwise_and)
    V.tensor_scalar(nbO[:], bass.AP(X8a.tensor, X8a.offset + 25,
                                    [list(X8a.ap[0]), [KP, FD], [1, 12]]),
                    4, None, ALU.logical_shift_right)
    xev = bass.AP(Xa.tensor, Xa.offset, [list(Xa.ap[0]), [K, FD], [2, 13]])
    xod = bass.AP(Xa.tensor, Xa.offset + 1, [list(Xa.ap[0]), [K, FD], [2, 12]])
    V.scalar_tensor_tensor(xev, nbE[:], STEP, xev, ALU.mult, ALU.add)
    V.scalar_tensor_tensor(xod, nbO[:], STEP, xod, ALU.mult, ALU.add)

    def ycol(c):                       # output scalar col c strided [128,FD]
        return mk(Ya, c, [[J, FD]])

    def ygrpA(c0):                     # chains 0,2,4 -> 3 joints stride 15
        return mk(Ya, c0, [[15, 3], [J, FD]])

    def ygrpB(c0):                     # chains 1,3 -> 2 joints stride 15
        return mk(Ya, c0, [[15, 2], [J, FD]])

    def srcA(t):                       # chain-major [128,5*FD] -> chains 0,2,4
        a = t if isinstance(t, bass.AP) else t[:]
        return bass.AP(a.tensor, a.offset, [list(a.ap[0]), [2 * FD, 3], [1, FD]])

    def srcB(t):                       # chains 1,3
        a = t if isinstance(t, bass.AP) else t[:]
        return bass.AP(a.tensor, a.offset + FD,
                       [list(a.ap[0]), [2 * FD, 2], [1, FD]])

    # ---------------- trig: 5 groups ----------------
    def trig(tag, xap, n):
        fd = n * FD
        u = scr.tile([128, fd], f16, tag="trigU", name="trigU")
        w = scr.tile([128, fd], f16, tag="trigW", name="trigW")
        A.activation(u[:], xap, AF.Sin, scale=0.5)
        A.activation(w[:], xap, AF.Sin, scale=0.25)
        q = scr.tile([128, fd], f16, tag="trigQ", name="trigQ")
        c = per.tile([128, fd], f16, tag=f"C{tag}", name=f"C{tag}")
        s = per.tile([128, fd], f16, tag=f"S{tag}", name=f"S{tag}")
        A.square(q[:], u[:])
        V.tensor_scalar(c[:], q[:], -2.0, 1.0, ALU.mult, ALU.add)
        A.square(q[:], w[:])
        V.tensor_scalar(q[:], q[:], -2.0, 1.0, ALU.mult, ALU.add)  # v in q
        V.scalar_tensor_tensor(s[:], u[:], 2.0, q[:], ALU.mult, ALU.mult)
        return c, s

    Cpt, Spt = trig("pt", mk(Xa, 0, [[1, 5], [K, FD]]), 5)
    CS = [trig(f"p{j}", mk(Xa, 5 + j, [[4, 5], [K, FD]]), 5) for j in range(4)]

    def pt(t, i):
        return t[:, i * FD:(i + 1) * FD]

    c0, s0 = pt(Cpt, 0), pt(Spt, 0)
    c1, s1 = pt(Cpt, 1), pt(Spt, 1)
    c2, s2 = pt(Cpt, 2), pt(Spt, 2)
    c3, s3 = pt(Cpt, 3), pt(Spt, 3)
    c4, s4 = pt(Cpt, 4), pt(Spt, 4)

    def tt(out, a, b, op):
        V.tensor_tensor(out, a, b, op)

    def fresh(tag, fd=FD, dt=f16, pool=None):
        return (pool or scr).tile([128, fd], dt, tag=tag, name=tag)

    def mul(a, b, tag="m", fd=FD):
        o = fresh(tag, fd=fd)
        tt(o[:], a, b, ALU.mult)
        return o[:]

    def nmul(a, b, tag="m"):           # -(a*b)
        o = fresh(tag)
        V.scalar_tensor_tensor(o[:], a, -1.0, b, ALU.mult, ALU.mult)
        return o[:]

    def comb(a, b, op, tag="m", pool=None, fd=FD):
        o = fresh(tag, fd=fd, pool=pool)
        tt(o[:], a, b, op)
        return o[:]

    # ---------------- pelvis R ----------------
    ms0s1 = mul(s0, s1, "ms01")
    mc0s1 = mul(c0, s1, "mc01")
    P1x = nmul(s0, c1, "P1x")
    P1y = mul(c0, c1, "P1y")
    P1z = s1                                        # alias
    P0x = comb(mul(c0, c2), mul(ms0s1, s2, "m2"), ALU.subtract, "P0x", per)
    P0y = comb(mul(s0, c2), mul(mc0s1, s2, "m2"), ALU.add, "P0y", per)
    P0z = nmul(c1, s2, "P0z")
    P2x = comb(mul(c0, s2), mul(ms0s1, c2, "m2"), ALU.add, "P2x", per)
    P2y = comb(mul(s0, s2), mul(mc0s1, c2, "m2"), ALU.subtract, "P2y", per)
    P2z = mul(c1, c2, "P2z")
    P0 = (P0x, P0y, P0z)
    P1 = (P1x, P1y, P1z)
    P2 = (P2x, P2y, P2z)

    # ---------------- torso R = Rpel @ Rz3 @ Ry4 ----------------
    def colupd(cc, ss, A3, B3, tagp, pool=None, fd=FD):
        """returns cc*A + ss*B per component."""
        out = []
        for i, (a, b) in enumerate(zip(A3, B3)):
            out.append(comb(mul(cc, a, "ca", fd), mul(ss, b, "cb", fd), ALU.add,
                            f"{tagp}{i}", pool, fd))
        return tuple(out)

    def colupd_sub(cc, ss, A3, B3, tagp, pool=None, fd=FD):
        """returns cc*A - ss*B per component."""
        out = []
        for i, (a, b) in enumerate(zip(A3, B3)):
            out.append(comb(mul(cc, a, "ca", fd), mul(ss, b, "cb", fd),
                            ALU.subtract, f"{tagp}{i}", pool, fd))
        return tuple(out)

    D0t = colupd(c3, s3, P0, P1, "D0t")
    D1t = colupd_sub(c3, s3, P1, P0, "D1t", per)       # E1 = D1t
    E0 = colupd_sub(c4, s4, D0t, P2, "E0", per)
    E2 = colupd(s4, c4, D0t, P2, "E2", per)

    # ---------------- phase A translations (unit scale) ----------------
    TP = [per.tile([128, FDC], f16, tag=f"TP{c}", name=f"TP{c}")
          for c in range(3)]

    def tp_slice(c, i):
        return TP[c][:, i * FD:(i + 1) * FD]

    QTOR = 127.0 / MARGIN            # S3*127/(S3*MARGIN) folded
    QHIP = 127.0 / MARGIN
    QSH = 127.0 / (B_SH * MARGIN)
    for c in range(3):
        # torso t = S3*D1 -> Y joint1 (quantized) + TP[neck]
        A.mul(ycol(0 + c), D1t[c], QTOR)             # D1t*S3*127/(S3*M)
        A.mul(tp_slice(c, 0), D1t[c], S3)
        # hips: +-S2*P0 -> TP legs; left hip -> Y
        A.mul(tp_slice(c, 1), P0[c], S2)
        A.mul(tp_slice(c, 2), P0[c], -S2)
        A.mul(ycol(9 + c), P0[c], QHIP)              # P0*S2*127/(S2*M)
        # shoulders: t_tor +- S8*E0 -> TP arms; left shoulder -> Y
        u = fresh("shu")
        A.mul(u[:], E0[c], S8)
        tt(tp_slice(c, 3), tp_slice(c, 0), u[:], ALU.add)
        tt(tp_slice(c, 4), tp_slice(c, 0), u[:], ALU.subtract)
        A.mul(ycol(24 + c), tp_slice(c, 3), QSH)

    # ---------------- batched parent-R tiles ----------------
    # chains: 0=neck(E), 1,2=legs(P), 3,4=arms(E)
    PR = [[per.tile([128, FDC], f16, tag=f"PR{c}{i}", name=f"PR{c}{i}")
           for i in range(3)] for c in range(3)]
    for ci, (Ecol, Pcol) in enumerate(((E0, P0), (D1t, P1), (E2, P2))):
        for i in range(3):
            dst = PR[ci][i][:]
            e = Ecol[i]
            p = Pcol[i]

            def bc2(src):
                return bass.AP(src.tensor, src.offset,
                               [list(src.ap[0]), [0, 2], [1, FD]])

            A.copy(mk(dst, 0, [[1, FD]]), e)
            A.copy(mk(dst, FD, [[1, 2 * FD]]), bc2(p))
            A.copy(mk(dst, 3 * FD, [[1, 2 * FD]]), bc2(e))

    def prc(c):
        return tuple(PR[c][i][:] for i in range(3))

    cA, sA = (t[:] for t in CS[0])
    cB, sB = (t[:] for t in CS[1])
    cG, sG = (t[:] for t in CS[2])
    cD, sD = (t[:] for t in CS[3])

    # ---------------- batched chain (FD=1280 ops) ----------------
    bD0 = colupd(cA, sA, prc(0), prc(1), "bD0", per, FDC)
    bD1 = colupd_sub(cA, sA, prc(1), prc(0), "bD1", per, FDC)
    bK1 = colupd(cB, sB, bD1, prc(2), "bK1", per, FDC)
    bK2 = colupd_sub(cB, sB, prc(2), bD1, "bK2", per, FDC)
    bK2p = colupd(sG, cG, bD0, bK2, "bD1", per, FDC)  # reuse bD1 slots
    bC1 = colupd(cD, sD, bK1, bK2p, "bD0", per, FDC)  # reuse bD0 slots

    # constant tiles: per-chain signed bone lengths and quant scales
    dT1 = fresh("dT1", FDC, pool=per)
    dT2 = fresh("dT2", FDC, pool=per)
    Qk = fresh("Qk", FDC, pool=per)
    Qd = fresh("Qd", FDC, pool=per)
    for i in range(5):
        sl = slice(i * FD, (i + 1) * FD)
        V.memset(dT1[:, sl], DT1[i])
        V.memset(dT2[:, sl], DT2[i])
        V.memset(Qk[:, sl], 127.0 / (BK[i] * MARGIN))
        V.memset(Qd[:, sl], 127.0 / (BD[i] * MARGIN))

    for c in range(3):
        u = fresh("btr", FDC)
        tt(u[:], dT1[:], bK1[c], ALU.mult)
        kn = fresh("kn", FDC)
        tt(kn[:], TP[c][:], u[:], ALU.add)               # knee-level joints
        u2 = fresh("btr2", FDC)
        tt(u2[:], dT2[:], bC1[c], ALU.mult)
        ds = fresh("ds", FDC)
        tt(ds[:], kn[:], u2[:], ALU.add)                 # distal joints
        knq = fresh("knq", FDC)
        tt(knq[:], kn[:], Qk[:], ALU.mult)               # quantize
        dsq = fresh("dsq", FDC)
        tt(dsq[:], ds[:], Qd[:], ALU.mult)
        A.copy(ygrpA(3 + c), srcA(knq))
        A.copy(ygrpB(12 + c), srcB(knq))
        A.copy(ygrpA(6 + c), srcA(dsq))
        A.copy(ygrpB(15 + c), srcB(dsq))

    HY = J * FD // 2
    for h in range(2):
        nc.gpsimd.dma_start(bass.AP(y.tensor, base * J + h * HY,
                                    [[FD * J, 128], [1, HY]]),
                            Y[:, h * HY:(h + 1) * HY])


# ---------------------------------------------------------------------------
# Cached PJRT runner: jit(shard_map(bass_exec)) built once; the previous
# call's device output buffers (already copied to host) are donated back as
# the custom-call output operands, so steady-state wire traffic is just
# x (f16 up) + y (f16 down).
# ---------------------------------------------------------------------------
_STATE = None


def _init():
    nc = build()
    b2j.install_neuronx_cc_hook()

    partition_name = (nc.partition_id_tensor.name
                      if nc.partition_id_tensor else None)
    in_names, out_names, out_avals = [], [], []
    for alloc in nc.m.functions[0].allocations:
        if not isinstance(alloc, mybir.MemoryLocationSet):
            continue
        name = alloc.memorylocations[0].name
        if alloc.kind == "ExternalInput":
            if name != partition_name:
                in_names.append(name)
        elif alloc.kind == "ExternalOutput":
            out_names.append(name)
            out_avals.append(jax.core.ShapedArray(
                tuple(alloc.tensor_shape), mybir.dt.np(alloc.dtype)))
    assert in_names == ["x"] and out_names == ["y"], (in_names, out_names)
    n_params = len(in_names)
    in_names_all = in_names + out_names
    if partition_name is not None:
        in_names_all.append(partition_name)
    donate = tuple(range(n_params, n_params + len(out_names)))

    def _body(*args):
        operands = list(args)
        if partition_name is not None:
            operands.append(b2j.partition_id_tensor())
        outs = b2j._bass_exec_p.bind(
            *operands,
            out_avals=tuple(out_avals),
            in_names=tuple(in_names_all),
            out_names=tuple(out_names),
            lowering_input_output_aliases=(),
            sim_require_finite=True,
            sim_require_nnan=True,
            nc=nc,
        )
        return tuple(outs)

    devices = jax.devices()[:NCORE]
    assert len(devices) == NCORE
    mesh = Mesh(np.asarray(devices), ("core",))
    nin = n_params + len(out_names)
    fn = jax.jit(
        shard_map(_body, mesh=mesh,
                  in_specs=(PartitionSpec("core"),) * nin,
                  out_specs=(PartitionSpec("core"),) * len(out_names),
                  check_rep=False),
        donate_argnums=donate,
        keep_unused=True,
    )
    # AOT-compile to trim per-call dispatch overhead.
    try:
        fn = fn.lower(jax.ShapeDtypeStruct((NG, KP), np.uint8),
                      jax.ShapeDtypeStruct((NG, J), np.int8)).compile()
    except Exception:
        pass
    return {"fn": fn, "prev": None}


_INV_STEP = np.float32(1.0 / STEP)
_QBIAS = np.float32(ABIAS / STEP + 0.5)   # +0.5: round via trunc


def _pack(xg):
    """Pack angle block (R,>=25) f32 -> (R,38) u8 of 12-bit fixed point."""
    t = xg[:, :25] * _INV_STEP
    t += _QBIAS
    np.clip(t, 0.0, 4095.0, out=t)
    v = t.astype(np.uint16)
    out = np.empty((t.shape[0], KP), np.uint8)
    out[:, :25] = v >> 4
    lo = v[:, 0:24:2] & 15
    hi = v[:, 1:25:2] & 15
    lo |= hi << 4
    out[:, 25:37] = lo
    out[:, 37] = v[:, 24] & 15
    return out


def _assemble(res, y8, scl):
    """Dequantize shipped [*,39] int8 block into final [*,51] f32 rows."""
    B = y8.astype(np.float32)
    B *= DEQ[None, :]
    B *= scl[:, None]
    res[:, 0:3] = 0.0                                   # pelvis
    res[:, 3:21] = B[:, 0:18]                           # j1..j6
    res[:, 24:30] = B[:, 18:24]                         # j8, j9
    res[:, 30:39] = B[:, 24:33]                         # j10, j11, j12
    res[:, 42:48] = B[:, 33:39]                         # j14, j15
    res[:, 21:24] = -B[:, 9:12]                         # rhip = -lhip
    res[:, 39:42] = 2.0 * B[:, 0:3] - B[:, 24:27]       # rsh = 2*torso - lsh
    res[:, 48:51] = 0.5 * (B[:, 15:18] + B[:, 18:21])   # thorax = (j6+j8)/2


def kernel(x: np.ndarray) -> np.ndarray:
    global _STATE
    if _STATE is None:
        _STATE = _init()
    st = _STATE

    x = np.asarray(x)
    if st["prev"] is None:
        st["prev"] = [np.zeros((NG, J), np.int8) for _ in range(NGRP)]

    # Dispatch group g, converting group g+1's input while g uploads and
    # queueing g's device->host copies right away.
    outs = []
    all_datas = []
    for g in range(NGRP):
        xg8 = _pack(x[g * NG:(g + 1) * NG])
        out, = st["fn"](xg8, st["prev"][g])
        outs.append(out)
        shards = sorted(out.addressable_shards,
                        key=lambda s: s.index[0].start or 0)
        datas = [s.data for s in shards]
        all_datas.extend(datas)
        for d in datas:
            try:
                d.copy_to_host_async()
            except Exception:
                pass

    # Assemble each shard's rows while later shards are still on the wire.
    scl = np.ascontiguousarray(x[:, 25], dtype=np.float32)
    res = np.empty((N, 51), np.float32)
    r0 = 0
    for d in all_datas:
        y8 = np.asarray(d)
        r1 = r0 + y8.shape[0]
        _assemble(res[r0:r1], y8, scl[r0:r1])
        r0 = r1
    assert r0 == N
    st["prev"] = outs                    # donate next call (already fetched)
    return res
